# revision 29
# baseline (speedup 1.0000x reference)
"""Trainium2 Bass kernel for: 3x3 conv (reflect pad) + BatchNorm + LeakyReLU + mask.

Input  x:    (1, 64, 512, 512) f32
       W:    (128, 64, 3, 3)   f32
       gamma/beta/mean/var: (128,) f32
       mask: (1, 128, 512, 512) int32 (0/1)
Output (1, 128, 512, 512) f32

Strategy (8 cores, SPMD):
  - Shard H spatially: core c computes output rows [64c, 64c+64).
  - Host reflect-pads x to (64, 514, 514), appends 2 duplicate rows, and ships
    each core TWO bf16 copies of its 67-row slab (second copy shifted down one
    row) stacked into a [128, 67*514] SBUF image. A K=128 matmul against
    stacked weights then computes two conv taps at once:
      partitions   0..63 : channel ci at row y+dy
      partitions 64..127 : channel ci at row y+dy+1
  - 9 taps -> 6 matmuls per output row: 3 "pair" matmuls (dy=0&1, dx=0..2)
    and 3 dy=2 matmuls whose lower 64 weight rows are zero.
  - PSUM accumulates fp32; epilogue = ACT Identity(psum*scale+shift),
    DVE max(z*0.01, z) for LeakyReLU, DVE multiply by uint8 mask.
"""

import numpy as np
import ml_dtypes

import concourse.bacc as bacc
import concourse.bass as bass
import concourse.mybir as mybir
import concourse.tile as tile
from concourse.bass_utils import run_bass_kernel_spmd

bf16 = ml_dtypes.bfloat16

N_CORES = 8
C_IN = 64
C_OUT = 128
H = 512
W_IMG = 512
HS = H // N_CORES            # 64 output rows per core
WP = W_IMG + 2               # 514 padded columns
NROW = HS + 3                # 67 rows per stacked copy
FREE = NROW * WP             # per-partition free elems of the x image
G = 8                        # output rows per pipeline group
LEAK = 0.01
EPS = 1e-5

_CACHE = {}
LAST_RESULTS = None          # BassKernelResults of the last run (for test.py)


def _build_program(hw_lrelu: bool = True) -> bass.Bass:
    """hw_lrelu=True uses the ACT engine's native Lrelu (not implemented in
    CoreSim); False uses an Identity + DVE max(z*a, z) fallback."""
    nc = bacc.Bacc("TRN2", target_bir_lowering=False, debug=False,
                   num_devices=N_CORES)
    f32 = mybir.dt.float32
    bf = mybir.dt.bfloat16
    u8 = mybir.dt.uint8

    xs_d = nc.dram_tensor("xs", [128, FREE], bf, kind="ExternalInput")
    wp_d = nc.dram_tensor("wp", [6, 128, C_OUT], bf, kind="ExternalInput")
    bn_d = nc.dram_tensor("bn", [C_OUT, 2], f32, kind="ExternalInput")
    mk_d = nc.dram_tensor("msk", [C_OUT, HS * W_IMG], u8, kind="ExternalInput")
    out_d = nc.dram_tensor("out", [C_OUT, HS * W_IMG], f32, kind="ExternalOutput")

    with tile.TileContext(nc) as tc:
        with tc.tile_pool(name="const", bufs=1) as cpool, \
             tc.tile_pool(name="xp", bufs=1) as xpool, \
             tc.tile_pool(name="mp", bufs=3) as mpool, \
             tc.tile_pool(name="zp", bufs=4) as zpool, \
             tc.tile_pool(name="op", bufs=3) as opool, \
             tc.tile_pool(name="ps", bufs=8, space="PSUM") as ppool:

            wts = [cpool.tile([128, C_OUT], bf, name=f"w{j}", tag=f"w{j}")
                   for j in range(6)]
            bn = cpool.tile([C_OUT, 2], f32, name="bn_t", tag="bn_t")
            xs = xpool.tile([128, FREE], bf, name="xs_t", tag="xs_t")

            def load_x(r0, r1, eng=None):
                # sync ring (qSPDynamicHW) carries the bulk x stream so
                # stores can never sit ahead of x chunks in its FIFO; the
                # earliest chunks also use the ACT ring (idle until ~25us)
                # for parallel landing
                (eng or nc.sync).dma_start(out=xs[:, r0 * WP:r1 * WP],
                                           in_=xs_d[:, r0 * WP:r1 * WP])

            # weights + bn go on the gpsimd queue so the HWDGE rings carry
            # only the x image early on
            for j in range(6):
                nc.gpsimd.dma_start(out=wts[j][:], in_=wp_d[j, :, :])
            nc.gpsimd.dma_start(out=bn[:], in_=bn_d[:])

            # fine-grained early chunks (land in parallel, PE starts early),
            # coarser later rows
            for i, r0 in enumerate(range(0, 16, 2)):
                load_x(r0, r0 + 2, nc.sync if i % 2 == 0 else nc.scalar)
            for i, r0 in enumerate(range(16, 48, 4)):
                load_x(r0, r0 + 4, nc.sync if i % 2 == 0 else nc.scalar)
            for r0 in range(48, NROW, 8):
                load_x(r0, min(NROW, r0 + 8))

            SG = 4                        # output rows per store tile
            mt = None
            ot = None

            def epilogue(y, pst):
                seg = slice((y % SG) * W_IMG, (y % SG + 1) * W_IMG)
                mseg = slice((y % G) * W_IMG, (y % G + 1) * W_IMG)
                if hw_lrelu:
                    nc.scalar.activation(
                        ot[:, seg], pst[:],
                        mybir.ActivationFunctionType.Lrelu,
                        bias=bn[:, 1:2], scale=bn[:, 0:1], alpha=LEAK)
                else:
                    zt = zpool.tile([C_OUT, W_IMG], f32, name="zt", tag="zt")
                    nc.scalar.activation(
                        zt[:], pst[:],
                        mybir.ActivationFunctionType.Identity,
                        bias=bn[:, 1:2], scale=bn[:, 0:1])
                    nc.vector.scalar_tensor_tensor(
                        ot[:, seg], zt[:], LEAK, zt[:],
                        op0=mybir.AluOpType.mult, op1=mybir.AluOpType.max)
                nc.vector.tensor_tensor(ot[:, seg], ot[:, seg], mt[:, mseg],
                                        op=mybir.AluOpType.mult)
                if y % SG == SG - 1:
                    d0 = (y - SG + 1) * W_IMG
                    # stores ride the ACT HWDGE ring (qActDynamicHW)
                    nc.scalar.dma_start(out=out_d[:, d0:d0 + SG * W_IMG],
                                        in_=ot[:])

            # rows processed in pairs: the three K=64 dy=2 taps of row y run
            # on PE rows 0-63 (copy0) while row y+1's run on rows 64-127
            # (copy1, same flat offset) — disjoint row-groups + separate PSUM
            # banks execute concurrently, so 2 rows cost 9 MM slots, not 12
            for y in range(0, HS, 2):
                if y % G == 0:
                    mt = mpool.tile([C_OUT, G * W_IMG], u8, name="mt", tag="mt")
                    # separate queue (gpsimd/SWDGE): masks must not wait
                    # behind the 8.8 MB x stream on the sync FIFO
                    nc.gpsimd.dma_start(
                        out=mt[:], in_=mk_d[:, y * W_IMG:(y + G) * W_IMG])
                if y % SG == 0:
                    ot = opool.tile([C_OUT, SG * W_IMG], f32, name="ot", tag="ot")
                ps_a = ppool.tile([C_OUT, W_IMG], f32, name="ps_a", tag="pst")
                ps_b = ppool.tile([C_OUT, W_IMG], f32, name="ps_b", tag="pst")
                for yy, ps in ((y, ps_a), (y + 1, ps_b)):
                    for dx in range(3):
                        off = yy * WP + dx
                        nc.tensor.matmul(ps[:], wts[dx][:],
                                         xs[:, off:off + W_IMG],
                                         start=(dx == 0), stop=False)
                for dx in range(3):
                    off = (y + 2) * WP + dx
                    nc.tensor.matmul(ps_a[:], wts[3 + dx][0:64, :],
                                     xs[0:64, off:off + W_IMG],
                                     start=False, stop=(dx == 2))
                    nc.tensor.matmul(ps_b[:], wts[3 + dx][64:128, :],
                                     xs[64:128, off:off + W_IMG],
                                     start=False, stop=(dx == 2))
                epilogue(y, ps_a)
                epilogue(y + 1, ps_b)
    nc.compile()
    return nc


def _get_program(hw_lrelu: bool = True) -> bass.Bass:
    key = ("nc", hw_lrelu)
    if key not in _CACHE:
        _CACHE[key] = _build_program(hw_lrelu)
    return _CACHE[key]


def make_in_maps(x, W, gamma, beta, mean, var, mask):
    """Host-side shard/pack of full inputs into per-core in_maps."""
    x = np.asarray(x, np.float32)
    W = np.asarray(W, np.float32)
    gamma = np.asarray(gamma, np.float32)
    beta = np.asarray(beta, np.float32)
    mean = np.asarray(mean, np.float32)
    var = np.asarray(var, np.float32)
    mask = np.asarray(mask)

    xp = np.pad(x[0], ((0, 0), (1, 1), (1, 1)), mode="reflect")   # [64,514,514]
    xpe = np.concatenate([xp, np.repeat(xp[:, -1:, :], 2, axis=1)], axis=1)
    xpb = xpe.astype(bf16)                                        # [64,516,514]

    wp = np.zeros((6, 128, C_OUT), np.float32)
    for dx in range(3):
        wp[dx, 0:64] = W[:, :, 0, dx].reshape(C_OUT, C_IN).T
        wp[dx, 64:128] = W[:, :, 1, dx].reshape(C_OUT, C_IN).T
        # dy=2 taps duplicated: rows 0-63 serve even rows via copy0,
        # rows 64-127 serve odd rows via copy1 (concurrent row-tiled MMs)
        wp[3 + dx, 0:64] = W[:, :, 2, dx].reshape(C_OUT, C_IN).T
        wp[3 + dx, 64:128] = W[:, :, 2, dx].reshape(C_OUT, C_IN).T
    wp = wp.astype(bf16)

    inv = 1.0 / np.sqrt(var + EPS)
    bn = np.stack([gamma * inv, beta - mean * gamma * inv],
                  axis=1).astype(np.float32)                      # [128,2]

    m8 = mask[0].astype(np.uint8)                                 # [128,512,512]

    in_maps = []
    for c in range(N_CORES):
        S = xpb[:, HS * c:HS * c + HS + 4, :]
        copy0 = np.ascontiguousarray(S[:, 0:NROW, :]).reshape(C_IN, FREE)
        copy1 = np.ascontiguousarray(S[:, 1:NROW + 1, :]).reshape(C_IN, FREE)
        xs_c = np.concatenate([copy0, copy1], axis=0)             # [128, FREE]
        mk_c = np.ascontiguousarray(
            m8[:, HS * c:HS * c + HS, :]).reshape(C_OUT, HS * W_IMG)
        in_maps.append(dict(xs=xs_c, wp=wp, bn=bn, msk=mk_c))
    return in_maps


def kernel(x, W, gamma, beta, mean, var, mask, _trace=False):
    global LAST_RESULTS
    nc = _get_program()
    in_maps = make_in_maps(x, W, gamma, beta, mean, var, mask)
    res = run_bass_kernel_spmd(nc, in_maps, list(range(N_CORES)), trace=_trace)
    LAST_RESULTS = res
    out = np.empty((1, C_OUT, H, W_IMG), np.float32)
    for c in range(N_CORES):
        out[0, :, HS * c:HS * c + HS, :] = \
            np.asarray(res.results[c]["out"]).reshape(C_OUT, HS, W_IMG)
    return out


# revision 30
# speedup vs baseline: 1.0361x; 1.0361x over previous
"""Trainium2 Bass kernel for: 3x3 conv (reflect pad) + BatchNorm + LeakyReLU + mask.

Input  x:    (1, 64, 512, 512) f32
       W:    (128, 64, 3, 3)   f32
       gamma/beta/mean/var: (128,) f32
       mask: (1, 128, 512, 512) int32 (0/1)
Output (1, 128, 512, 512) f32

Strategy (8 cores, SPMD):
  - Shard H spatially: core c computes output rows [64c, 64c+64).
  - Host reflect-pads x to (64, 514, 514), appends 2 duplicate rows, and ships
    each core TWO bf16 copies of its 67-row slab (second copy shifted down one
    row) stacked into a [128, 67*514] SBUF image. A K=128 matmul against
    stacked weights then computes two conv taps at once:
      partitions   0..63 : channel ci at row y+dy
      partitions 64..127 : channel ci at row y+dy+1
  - 9 taps -> 6 matmuls per output row: 3 "pair" matmuls (dy=0&1, dx=0..2)
    and 3 dy=2 matmuls whose lower 64 weight rows are zero.
  - PSUM accumulates fp32; epilogue = ACT Identity(psum*scale+shift),
    DVE max(z*0.01, z) for LeakyReLU, DVE multiply by uint8 mask.
"""

import numpy as np
import ml_dtypes

import concourse.bacc as bacc
import concourse.bass as bass
import concourse.mybir as mybir
import concourse.tile as tile
from concourse.bass_utils import run_bass_kernel_spmd

bf16 = ml_dtypes.bfloat16

N_CORES = 8
C_IN = 64
C_OUT = 128
H = 512
W_IMG = 512
HS = H // N_CORES            # 64 output rows per core
WP = W_IMG + 2               # 514 padded columns
NROW = HS + 3                # 67 rows per stacked copy
FREE = NROW * WP             # per-partition free elems of the x image
G = 8                        # output rows per pipeline group
LEAK = 0.01
EPS = 1e-5

_CACHE = {}
LAST_RESULTS = None          # BassKernelResults of the last run (for test.py)


def _build_program(hw_lrelu: bool = True) -> bass.Bass:
    """hw_lrelu=True uses the ACT engine's native Lrelu (not implemented in
    CoreSim); False uses an Identity + DVE max(z*a, z) fallback."""
    nc = bacc.Bacc("TRN2", target_bir_lowering=False, debug=False,
                   num_devices=N_CORES)
    f32 = mybir.dt.float32
    bf = mybir.dt.bfloat16
    u8 = mybir.dt.uint8

    xs_d = nc.dram_tensor("xs", [128, FREE], bf, kind="ExternalInput")
    wp_d = nc.dram_tensor("wp", [6, 128, C_OUT], bf, kind="ExternalInput")
    bn_d = nc.dram_tensor("bn", [C_OUT, 2], f32, kind="ExternalInput")
    mk_d = nc.dram_tensor("msk", [C_OUT, HS * W_IMG], u8, kind="ExternalInput")
    out_d = nc.dram_tensor("out", [C_OUT, HS * W_IMG], f32, kind="ExternalOutput")

    with tile.TileContext(nc) as tc:
        with tc.tile_pool(name="const", bufs=1) as cpool, \
             tc.tile_pool(name="xp", bufs=1) as xpool, \
             tc.tile_pool(name="mp", bufs=3) as mpool, \
             tc.tile_pool(name="zp", bufs=4) as zpool, \
             tc.tile_pool(name="op", bufs=3) as opool, \
             tc.tile_pool(name="ps", bufs=8, space="PSUM") as ppool:

            wts = [cpool.tile([128, C_OUT], bf, name=f"w{j}", tag=f"w{j}")
                   for j in range(6)]
            bn = cpool.tile([C_OUT, 2], f32, name="bn_t", tag="bn_t")
            xs = xpool.tile([128, FREE], bf, name="xs_t", tag="xs_t")

            def load_x(r0, r1, eng=None):
                # sync ring (qSPDynamicHW) carries the bulk x stream so
                # stores can never sit ahead of x chunks in its FIFO; the
                # earliest chunks also use the ACT ring (idle until ~25us)
                # for parallel landing
                (eng or nc.sync).dma_start(out=xs[:, r0 * WP:r1 * WP],
                                           in_=xs_d[:, r0 * WP:r1 * WP])

            # weights + bn go on the gpsimd queue so the HWDGE rings carry
            # only the x image early on
            for j in range(6):
                nc.gpsimd.dma_start(out=wts[j][:], in_=wp_d[j, :, :])
            nc.gpsimd.dma_start(out=bn[:], in_=bn_d[:])

            # fine-grained early chunks (land in parallel, PE starts early),
            # coarser later rows
            for i, r0 in enumerate(range(0, 16, 2)):
                load_x(r0, r0 + 2, nc.sync if i % 2 == 0 else nc.scalar)
            for i, r0 in enumerate(range(16, 32, 4)):
                load_x(r0, r0 + 4, nc.sync if i % 2 == 0 else nc.scalar)
            for r0 in range(32, NROW, 8):
                load_x(r0, min(NROW, r0 + 8))

            SG = 4                        # output rows per store tile
            mt = None
            ot = None

            def epilogue(y, pst):
                seg = slice((y % SG) * W_IMG, (y % SG + 1) * W_IMG)
                mseg = slice((y % G) * W_IMG, (y % G + 1) * W_IMG)
                if hw_lrelu:
                    nc.scalar.activation(
                        ot[:, seg], pst[:],
                        mybir.ActivationFunctionType.Lrelu,
                        bias=bn[:, 1:2], scale=bn[:, 0:1], alpha=LEAK)
                else:
                    zt = zpool.tile([C_OUT, W_IMG], f32, name="zt", tag="zt")
                    nc.scalar.activation(
                        zt[:], pst[:],
                        mybir.ActivationFunctionType.Identity,
                        bias=bn[:, 1:2], scale=bn[:, 0:1])
                    nc.vector.scalar_tensor_tensor(
                        ot[:, seg], zt[:], LEAK, zt[:],
                        op0=mybir.AluOpType.mult, op1=mybir.AluOpType.max)
                nc.vector.tensor_tensor(ot[:, seg], ot[:, seg], mt[:, mseg],
                                        op=mybir.AluOpType.mult)
                if y % SG == SG - 1:
                    d0 = (y - SG + 1) * W_IMG
                    # stores ride the ACT HWDGE ring (qActDynamicHW)
                    nc.scalar.dma_start(out=out_d[:, d0:d0 + SG * W_IMG],
                                        in_=ot[:])

            # rows processed in pairs: the three K=64 dy=2 taps of row y run
            # on PE rows 0-63 (copy0) while row y+1's run on rows 64-127
            # (copy1, same flat offset) — disjoint row-groups + separate PSUM
            # banks execute concurrently, so 2 rows cost 9 MM slots, not 12
            for y in range(0, HS, 2):
                if y % G == 0:
                    mt = mpool.tile([C_OUT, G * W_IMG], u8, name="mt", tag="mt")
                    # separate queue (gpsimd/SWDGE): masks must not wait
                    # behind the 8.8 MB x stream on the sync FIFO
                    nc.gpsimd.dma_start(
                        out=mt[:], in_=mk_d[:, y * W_IMG:(y + G) * W_IMG])
                if y % SG == 0:
                    ot = opool.tile([C_OUT, SG * W_IMG], f32, name="ot", tag="ot")
                ps_a = ppool.tile([C_OUT, W_IMG], f32, name="ps_a", tag="pst")
                ps_b = ppool.tile([C_OUT, W_IMG], f32, name="ps_b", tag="pst")
                for yy, ps in ((y, ps_a), (y + 1, ps_b)):
                    for dx in range(3):
                        off = yy * WP + dx
                        nc.tensor.matmul(ps[:], wts[dx][:],
                                         xs[:, off:off + W_IMG],
                                         start=(dx == 0), stop=False)
                for dx in range(3):
                    off = (y + 2) * WP + dx
                    nc.tensor.matmul(ps_a[:], wts[3 + dx][0:64, :],
                                     xs[0:64, off:off + W_IMG],
                                     start=False, stop=(dx == 2))
                    nc.tensor.matmul(ps_b[:], wts[3 + dx][64:128, :],
                                     xs[64:128, off:off + W_IMG],
                                     start=False, stop=(dx == 2))
                epilogue(y, ps_a)
                epilogue(y + 1, ps_b)
    nc.compile()
    return nc


def _get_program(hw_lrelu: bool = True) -> bass.Bass:
    key = ("nc", hw_lrelu)
    if key not in _CACHE:
        _CACHE[key] = _build_program(hw_lrelu)
    return _CACHE[key]


def make_in_maps(x, W, gamma, beta, mean, var, mask):
    """Host-side shard/pack of full inputs into per-core in_maps."""
    x = np.asarray(x, np.float32)
    W = np.asarray(W, np.float32)
    gamma = np.asarray(gamma, np.float32)
    beta = np.asarray(beta, np.float32)
    mean = np.asarray(mean, np.float32)
    var = np.asarray(var, np.float32)
    mask = np.asarray(mask)

    xp = np.pad(x[0], ((0, 0), (1, 1), (1, 1)), mode="reflect")   # [64,514,514]
    xpe = np.concatenate([xp, np.repeat(xp[:, -1:, :], 2, axis=1)], axis=1)
    xpb = xpe.astype(bf16)                                        # [64,516,514]

    wp = np.zeros((6, 128, C_OUT), np.float32)
    for dx in range(3):
        wp[dx, 0:64] = W[:, :, 0, dx].reshape(C_OUT, C_IN).T
        wp[dx, 64:128] = W[:, :, 1, dx].reshape(C_OUT, C_IN).T
        # dy=2 taps duplicated: rows 0-63 serve even rows via copy0,
        # rows 64-127 serve odd rows via copy1 (concurrent row-tiled MMs)
        wp[3 + dx, 0:64] = W[:, :, 2, dx].reshape(C_OUT, C_IN).T
        wp[3 + dx, 64:128] = W[:, :, 2, dx].reshape(C_OUT, C_IN).T
    wp = wp.astype(bf16)

    inv = 1.0 / np.sqrt(var + EPS)
    bn = np.stack([gamma * inv, beta - mean * gamma * inv],
                  axis=1).astype(np.float32)                      # [128,2]

    m8 = mask[0].astype(np.uint8)                                 # [128,512,512]

    in_maps = []
    for c in range(N_CORES):
        S = xpb[:, HS * c:HS * c + HS + 4, :]
        copy0 = np.ascontiguousarray(S[:, 0:NROW, :]).reshape(C_IN, FREE)
        copy1 = np.ascontiguousarray(S[:, 1:NROW + 1, :]).reshape(C_IN, FREE)
        xs_c = np.concatenate([copy0, copy1], axis=0)             # [128, FREE]
        mk_c = np.ascontiguousarray(
            m8[:, HS * c:HS * c + HS, :]).reshape(C_OUT, HS * W_IMG)
        in_maps.append(dict(xs=xs_c, wp=wp, bn=bn, msk=mk_c))
    return in_maps


def kernel(x, W, gamma, beta, mean, var, mask, _trace=False):
    global LAST_RESULTS
    nc = _get_program()
    in_maps = make_in_maps(x, W, gamma, beta, mean, var, mask)
    res = run_bass_kernel_spmd(nc, in_maps, list(range(N_CORES)), trace=_trace)
    LAST_RESULTS = res
    out = np.empty((1, C_OUT, H, W_IMG), np.float32)
    for c in range(N_CORES):
        out[0, :, HS * c:HS * c + HS, :] = \
            np.asarray(res.results[c]["out"]).reshape(C_OUT, HS, W_IMG)
    return out
